# revision 4
# baseline (speedup 1.0000x reference)
"""Trainium2 Bass kernel for nn_Autoencoder (batched OMP sparse coding).

Problem: L=32 layers, M=128, N=2048 dictionary atoms, S=8 sparsity, B=512.
reference computes, per layer l and token b, an S=8-step Orthogonal Matching
Pursuit of k[b,l,:] over dictionary D[l], then scatters coefficients into the
dense code y (B,L,N), decodes k_hat = D @ y and returns
(loss=mean((k_hat-k)^2), k_hat, y).

Sharding: layers across the 8 cores (4 each, dictionary slice stays SBUF
resident); within a core, B=512 tokens as 4 partition-chunks of 128.

Per (layer, chunk, step) on device:
  - corr = r @ D_l on PE (fp32, stationary = transposed residual chunk)
  - corr^2 + running max via DVE tensor_tensor_reduce straight from PSUM
  - argmax via DVE max_index (exact first-occurrence semantics)
  - selected atoms fetched with one batched gpsimd dma_gather from a
    host-pretransposed copy of D (atom-major), using index lists built
    on-device with a small mask matmul (wrapped int16 layout)
  - solve via unnormalized classical Gram-Schmidt (no sqrt needed: the
    R'x = alpha' system cancels the norms), coefficient back-substitution
  - y written with a multi-offset indirect DMA scatter into the pre-zeroed
    output buffer; k_hat = kt - r_final; loss accumulated on device.
"""
import numpy as np

P = 128
N = 2048
M = 128
NCHUNK = 4
NL = 4          # layers per core
NS = 8          # sparsity / OMP steps
NCORES = 8
L = NL * NCORES
B = NCHUNK * P

_cached = {}


def _host_consts():
    b = np.arange(P)
    M1 = (b[:, None] % 16 == b[None, :] % 16).astype(np.float32)
    M2 = (b[:, None] // 16 == np.arange(8)[None, :]).astype(np.float32)
    ones8 = np.ones((P, 8), dtype=np.float32)
    return M1, M2, ones8


def _build():
    import concourse.bacc as bacc
    import concourse.tile as tile
    from concourse import mybir
    from concourse.bass import IndirectOffsetOnAxis
    from concourse.masks import make_identity

    F = mybir.dt.float32
    AL = mybir.AluOpType

    nc = bacc.Bacc("TRN2", target_bir_lowering=False, num_swdge_queues=1)

    Dc = nc.dram_tensor("Dc", [NL, M, N], F, kind="ExternalInput")
    DTc = nc.dram_tensor("DTc", [NL, N, M], F, kind="ExternalInput")
    kc = nc.dram_tensor("kc", [B, NL, M], F, kind="ExternalInput")
    M1c = nc.dram_tensor("M1c", [P, P], F, kind="ExternalInput")
    M2c = nc.dram_tensor("M2c", [P, 8], F, kind="ExternalInput")
    ONES8 = nc.dram_tensor("ONES8", [P, 8], F, kind="ExternalInput")

    yout = nc.dram_tensor("yout", [B * NL * N, 1], F, kind="ExternalOutput")
    khat = nc.dram_tensor("khat", [B, NL, M], F, kind="ExternalOutput")
    losspart = nc.dram_tensor("losspart", [P, 1], F, kind="ExternalOutput")

    with tile.TileContext(nc) as tc:
        with tc.tile_pool(name="consts", bufs=1) as consts, \
             tc.tile_pool(name="dp", bufs=1) as dp, \
             tc.tile_pool(name="state", bufs=1) as state, \
             tc.tile_pool(name="ktp", bufs=2) as ktp, \
             tc.tile_pool(name="csqp", bufs=3) as csqp, \
             tc.tile_pool(name="scr", bufs=8) as scr, \
             tc.tile_pool(name="tiny", bufs=24) as tiny, \
             tc.tile_pool(name="cps", bufs=3, space="PSUM") as cps, \
             tc.tile_pool(name="sps", bufs=2, space="PSUM") as sps:

            ident = consts.tile([P, P], F)
            make_identity(nc, ident[:])
            m1 = consts.tile([P, P], F)
            nc.sync.dma_start(m1[:], M1c[:])
            m2 = consts.tile([P, 8], F)
            nc.sync.dma_start(m2[:], M2c[:])
            ones8 = consts.tile([P, 8], F)
            nc.sync.dma_start(ones8[:], ONES8[:])

            dmats = []
            for l in range(NL):
                dm = dp.tile([P, N], F, tag=f"d{l}")
                nc.sync.dma_start(dm[:], Dc[l])
                dmats.append(dm)

            lp = state.tile([P, 1], F, tag="losspart")
            nc.vector.memset(lp[:], 0.0)

            for l in range(NL):
                dmat = dmats[l]
                kt = ktp.tile([P, NCHUNK, M], F, tag="kt")
                nc.sync.dma_start(
                    kt[:], kc[:, l, :].rearrange("(c p) m -> p c m", p=P))
                r = state.tile([P, NCHUNK, M], F, tag="r")
                nc.vector.tensor_copy(r[:], kt[:])
                U = state.tile([P, NS, NCHUNK, M], F, tag="U")
                UH = state.tile([P, NS, NCHUNK, M], F, tag="UH")
                Rco = state.tile([P, NCHUNK, NS * 8], F, tag="Rco")
                alpha = state.tile([P, NCHUNK, NS], F, tag="alpha")
                nu = state.tile([P, NCHUNK, NS], F, tag="nu")
                nsel = state.tile([P, NCHUNK, NS], F, tag="nsel")

                for t in range(NS):
                    rhsb = tiny.tile([P, 32], F, tag="rhsb")
                    for c in range(NCHUNK):
                        rt_ps = sps.tile([P, P], F, tag="sp")
                        nc.tensor.transpose(rt_ps[:], r[:, c, :], ident[:])
                        rt = scr.tile([P, P], F, tag="rt")
                        nc.scalar.copy(rt[:], rt_ps[:])
                        csq = csqp.tile([P, N], F, tag="csq")
                        m2h = tiny.tile([P, 2], F, tag="m2h")
                        for h in range(2):
                            cp = cps.tile([P, N // 2], F, tag="cp")
                            for q in range(2):
                                nc.tensor.matmul(
                                    cp[:, q * 512:(q + 1) * 512], rt[:],
                                    dmat[:, (2 * h + q) * 512:(2 * h + q + 1) * 512],
                                    start=True, stop=True)
                            nc.vector.tensor_reduce(
                                m2h[:, h:h + 1], cp[:],
                                mybir.AxisListType.X, AL.max,
                                apply_absolute_value=True)
                            nc.scalar.activation(
                                csq[:, h * (N // 2):(h + 1) * (N // 2)], cp[:],
                                mybir.ActivationFunctionType.Abs)
                        m2v = tiny.tile([P, 1], F, tag="m2v")
                        nc.vector.tensor_tensor(
                            m2v[:], m2h[:, 0:1], m2h[:, 1:2], op=AL.max)
                        m8 = tiny.tile([P, 8], F, tag="m8")
                        nc.vector.tensor_scalar(
                            out=m8[:], in0=ones8[:], scalar1=m2v[:],
                            scalar2=None, op0=AL.mult)
                        n8 = tiny.tile([P, 8], mybir.dt.uint32, tag="n8")
                        nc.vector.max_index(n8[:], m8[:], csq[:])
                        nc.vector.tensor_copy(nsel[:, c, t:t + 1], n8[:, 0:1])
                        nc.vector.tensor_scalar(
                            out=rhsb[:, c * 8:(c + 1) * 8], in0=m2[:],
                            scalar1=nsel[:, c, t:t + 1], scalar2=None,
                            op0=AL.mult)

                    wps = sps.tile([P, P], F, tag="sp")
                    nc.tensor.matmul(wps[:, 0:32], m1[:], rhsb[:],
                                     start=True, stop=True)
                    widx = tiny.tile([P, 32], mybir.dt.int16, tag="widx")
                    nc.vector.tensor_copy(widx[:], wps[:, 0:32])
                    nc.gpsimd.dma_gather(
                        U[:, t], DTc[l], widx[:], num_idxs=B,
                        num_idxs_reg=B, elem_size=M, queue_num=0)

                    for c in range(NCHUNK):
                        d_t = U[:, t, c, :]
                        for i in range(t):
                            prod = scr.tile([P, M], F, tag="prod")
                            nc.gpsimd.tensor_tensor(
                                prod[:], U[:, i, c, :], d_t, op=AL.mult)
                            nc.scalar.activation(
                                prod[:], prod[:],
                                mybir.ActivationFunctionType.Copy,
                                accum_out=Rco[:, c, t * 8 + i:t * 8 + i + 1])
                        for i in range(t):
                            sc = scr.tile([P, M], F, tag="sc")
                            nc.scalar.mul(sc[:], UH[:, i, c, :],
                                          Rco[:, c, t * 8 + i:t * 8 + i + 1])
                            nc.gpsimd.tensor_tensor(
                                d_t, d_t, sc[:], op=AL.subtract)
                        prod2 = scr.tile([P, M], F, tag="prod")
                        nc.gpsimd.tensor_tensor(prod2[:], d_t, d_t, op=AL.mult)
                        nc.scalar.activation(
                            prod2[:], prod2[:],
                            mybir.ActivationFunctionType.Copy,
                            accum_out=Rco[:, c, t * 8 + t:t * 8 + t + 1])
                        nc.vector.reciprocal(nu[:, c, t:t + 1],
                                             Rco[:, c, t * 8 + t:t * 8 + t + 1])
                        nc.scalar.mul(UH[:, t, c, :], d_t, nu[:, c, t:t + 1])
                        prod3 = scr.tile([P, M], F, tag="prod")
                        nc.gpsimd.tensor_tensor(prod3[:], d_t, r[:, c, :],
                                                op=AL.mult)
                        nc.scalar.activation(
                            prod3[:], prod3[:],
                            mybir.ActivationFunctionType.Copy,
                            accum_out=alpha[:, c, t:t + 1])
                        sc2 = scr.tile([P, M], F, tag="sc")
                        nc.scalar.mul(sc2[:], UH[:, t, c, :], alpha[:, c, t:t + 1])
                        nc.gpsimd.tensor_tensor(
                            r[:, c, :], r[:, c, :], sc2[:], op=AL.subtract)

                x = state.tile([P, NCHUNK, NS], F, tag="x")
                for c in range(NCHUNK):
                    for t in range(NS - 1, -1, -1):
                        tmp = tiny.tile([P, 1], F, tag="bs")
                        if t < NS - 1:
                            s = tiny.tile([P, 1], F, tag="bs2")
                            junk = tiny.tile([P, NS - 1 - t], F, tag="bsj")
                            nc.vector.tensor_tensor(
                                junk[:],
                                Rco[:, c, (t + 1) * 8 + t::8],
                                x[:, c, t + 1:NS], op=AL.mult)
                            nc.vector.tensor_reduce(
                                s[:], junk[:], mybir.AxisListType.X, AL.add)
                            nc.vector.tensor_tensor(
                                tmp[:], alpha[:, c, t:t + 1], s[:],
                                op=AL.subtract)
                        else:
                            nc.vector.tensor_copy(tmp[:], alpha[:, c, t:t + 1])
                        nc.vector.tensor_tensor(
                            x[:, c, t:t + 1], tmp[:], nu[:, c, t:t + 1],
                            op=AL.mult)

                kh = ktp.tile([P, NCHUNK, M], F, tag="kh")
                nc.gpsimd.tensor_tensor(kh[:], kt[:], r[:], op=AL.subtract)
                nc.sync.dma_start(
                    khat[:, l, :].rearrange("(c p) m -> p c m", p=P), kh[:])
                ljunk = scr.tile([P, NCHUNK * M], F, tag="lj")
                lpart = tiny.tile([P, 1], F, tag="lpart")
                nc.scalar.activation(
                    ljunk[:], r[:].rearrange("p c m -> p (c m)"),
                    mybir.ActivationFunctionType.Square,
                    accum_out=lpart[:])
                nc.vector.tensor_tensor(lp[:], lp[:], lpart[:], op=AL.add)

                for c in range(NCHUNK):
                    offi0 = tiny.tile([P, NS], mybir.dt.int32, tag="offi0")
                    nc.gpsimd.iota(
                        offi0[:], pattern=[[0, NS]],
                        base=(c * P * NL + l) * N,
                        channel_multiplier=NL * N)
                    offf = tiny.tile([P, NS], F, tag="offf")
                    nc.vector.tensor_copy(offf[:], offi0[:])
                    nc.vector.tensor_tensor(
                        offf[:], offf[:], nsel[:, c, :], op=AL.add)
                    offi = tiny.tile([P, NS], mybir.dt.int32, tag="offi")
                    nc.vector.tensor_copy(offi[:], offf[:])
                    for si in range(NS):
                        nc.gpsimd.indirect_dma_start(
                            out=yout[:],
                            out_offset=IndirectOffsetOnAxis(
                                ap=offi[:, si:si + 1], axis=0),
                            in_=x[:, c, si:si + 1], in_offset=None)

            nc.sync.dma_start(losspart[:], lp[:])

    nc.compile()
    return nc


def _get_nc():
    if "nc" not in _cached:
        _cached["nc"] = _build()
    return _cached["nc"]


def kernel(D: np.ndarray, k: np.ndarray):
    """Full-input OMP autoencoder. D (32,128,2048), k (512,32,128) fp32.
    Returns (loss, k_hat (512,32,128), y (512,32,2048)) like reference()."""
    from concourse.bass_utils import run_bass_kernel_spmd

    D = np.ascontiguousarray(D, dtype=np.float32)
    k = np.ascontiguousarray(k, dtype=np.float32)
    M1, M2, ones8 = _host_consts()

    nc = _get_nc()
    in_maps = []
    for i in range(NCORES):
        Dcore = np.ascontiguousarray(D[i * NL:(i + 1) * NL])
        in_maps.append(dict(
            Dc=Dcore,
            DTc=np.ascontiguousarray(Dcore.transpose(0, 2, 1)),
            kc=np.ascontiguousarray(k[:, i * NL:(i + 1) * NL, :]),
            M1c=M1, M2c=M2, ONES8=ones8))

    res = run_bass_kernel_spmd(nc, in_maps, core_ids=list(range(NCORES)))
    outs = res.results

    k_hat = np.concatenate([o["khat"] for o in outs], axis=1)
    y = np.concatenate(
        [o["yout"].reshape(B, NL, N) for o in outs], axis=1)
    loss_total = sum(float(o["losspart"].sum()) for o in outs)
    loss = np.float32(loss_total / (B * L * M))
    return loss, k_hat, y
